# revision 4
# baseline (speedup 1.0000x reference)
"""CrystalDecoder Trainium2 kernel — 8-core data-parallel.

Strategy:
  * recon_node: per-core slice of 8192 nodes, feature-major MLP chain on
    TensorE/ScalarE. The ragged per-graph z broadcast is done gather-free via a
    telescoping "staircase" matmul: z-contribution = dQ @ S with
    S[g,i] = (seg[i] >= g) built on-device from a DVE compare (segment_ids
    sorted), dQ[g] = Q[g]-Q[g-1], Q = z_proj @ W_nd1.
  * recon_edge: depends only on the graph pair (gs,gd). Edges are sharded by
    gs-block (32 graphs/core); each core builds its 8192-entry pair table
    T3[(gs,gd)] = relu(P_top[gs]+P_bot[gd]+b) @ W_ed2 + b and the per-edge
    output is a dma_gather of 256B rows by sorted pair index.
  * graph heads: tiny, computed on every core; host takes core 0's.
All math in fp32 (except exact small-int compares carried in bf16).
"""

import sys

sys.path.insert(0, "/opt/trn_rl_repo")

import numpy as np
import ml_dtypes

import concourse.bacc as bacc
import concourse.mybir as mybir
from concourse.bass_utils import run_bass_kernel_spmd
from concourse.tile import TileContext

F32 = mybir.dt.float32
BF16 = mybir.dt.bfloat16
I16 = mybir.dt.int16
RELU = mybir.ActivationFunctionType.Relu

NCORES = 8
B, L, H, D2, NE = 256, 64, 128, 256, 1024
N, E = 65536, 524288
NODES_PER = N // NCORES          # 8192
CHUNK = 512
NCHUNK = NODES_PER // CHUNK      # 16
GS_PER = B // NCORES             # 32 graphs per core for the edge shard
NPAIR = GS_PER * B               # 8192 pair-table rows per core
TW = 64                          # pair-table row width in f32 (256B, dma_gather min)
EDGE_CAP = 69632                 # per-core edge capacity (mean 65536, +17 sigma)
GCH = 8704                       # edges per dma_gather call
NGATHER = EDGE_CAP // GCH        # 8
GCHB = GCH // 128                # 68 blocks per gather call
NBLK = EDGE_CAP // 128           # 544

_cached = {}


def _build():
    nc = bacc.Bacc("TRN2", target_bir_lowering=False, debug=False,
                   num_devices=NCORES)

    def din(name, shape, dt=F32):
        return nc.dram_tensor(name, shape, dt, kind="ExternalInput")

    def dout(name, shape, dt=F32):
        return nc.dram_tensor(name, shape, dt, kind="ExternalOutput")

    ne_d = din("ne", [NCHUNK, 128, 8, CHUNK])      # node_emb.T slice, chunk-major
    segb_d = din("segb", [128, NODES_PER], BF16)   # segment ids, bcast, bf16
    zl_d = din("zl", [L + 6, B])                   # [z.T ; lattice bcast]
    zmy_d = din("zmy", [L, GS_PER])                # z.T cols of this core's gs block
    eidx_d = din("eidx", [128, EDGE_CAP // 16], I16)
    iota_d = din("iota", [128, 2])
    ident_d = din("ident", [128, 128])

    wlat_d = din("wlat", [L, D2]);    blat_d = din("blat", [D2])
    wnep_d = din("wnep", [NE, D2]);   bnep_d = din("bnep", [D2])
    wnd1_d = din("wnd1", [D2, H]);    bnd1_d = din("bnd1", [H])
    wnd2_d = din("wnd2", [H, 4]);     bnd2_d = din("bnd2", [4])
    wtop_d = din("wtop", [D2, H])
    wbot_d = din("wbot", [D2, H]);    bed1_d = din("bed1", [H])
    wed2_d = din("wed2", [H, 3]);     bed2_d = din("bed2", [3])
    wen1_d = din("wen1", [L + 6, H]); ben1_d = din("ben1", [H])
    wen2_d = din("wen2", [H, 2]);     ben2_d = din("ben2", [2])
    wst1_d = din("wst1", [L + 6, H]); bst1_d = din("bst1", [H])
    wst2_d = din("wst2", [H, 9]);     bst2_d = din("bst2", [9])

    rn_d = dout("rn", [4, NODES_PER])
    re_d = dout("re", [128, NBLK * 3])
    pe_d = dout("pe", [2, B])
    ps_d = dout("ps", [9, B])

    with TileContext(nc) as tc:
        with (
            tc.tile_pool(name="const", bufs=1) as cp,
            tc.tile_pool(name="dram", bufs=1, space="DRAM") as dp,
            tc.tile_pool(name="work", bufs=2) as wp,
            tc.tile_pool(name="nein", bufs=2) as nep_pool,
            tc.tile_pool(name="acts", bufs=3) as ap,
            tc.tile_pool(name="gath", bufs=2) as gp,
            tc.tile_pool(name="pmisc", bufs=2, space="PSUM") as pmisc,
            tc.tile_pool(name="p1", bufs=3, space="PSUM") as p1p,
            tc.tile_pool(name="p2", bufs=2, space="PSUM") as p2p,
            tc.tile_pool(name="p4", bufs=1, space="PSUM") as p4p,
        ):
            # ---------- constant loads ----------
            def ld(pool, dram_ap, shape, dt=F32, tag=None):
                t = pool.tile(shape, dt, name=tag, tag=tag)
                nc.sync.dma_start(t[:], dram_ap)
                return t

            zl = ld(cp, zl_d.ap(), [L + 6, B], tag="zl")
            zmy = ld(cp, zmy_d.ap(), [L, GS_PER], tag="zmy")
            segb = ld(cp, segb_d.ap(), [128, NODES_PER], BF16, tag="segb")
            eidx = ld(cp, eidx_d.ap(), [128, EDGE_CAP // 16], I16, tag="eidx")
            iota = ld(cp, iota_d.ap(), [128, 2], tag="iota")
            ident = ld(cp, ident_d.ap(), [128, 128], tag="ident")

            wlat = ld(cp, wlat_d.ap(), [L, D2], tag="wlat")
            wnep = ld(cp, wnep_d.ap().rearrange("(k p) f -> p k f", p=128),
                      [128, 8, D2], tag="wnep")
            wnd1 = ld(cp, wnd1_d.ap().rearrange("(k p) f -> p k f", p=128),
                      [128, 2, H], tag="wnd1")
            wnd2 = ld(cp, wnd2_d.ap(), [H, 4], tag="wnd2")
            wtop = ld(cp, wtop_d.ap().rearrange("(k p) f -> p k f", p=128),
                      [128, 2, H], tag="wtop")
            wbot = ld(cp, wbot_d.ap().rearrange("(k p) f -> p k f", p=128),
                      [128, 2, H], tag="wbot")
            wed2 = ld(cp, wed2_d.ap(), [H, 3], tag="wed2")
            wen1 = ld(cp, wen1_d.ap(), [L + 6, H], tag="wen1")
            wen2 = ld(cp, wen2_d.ap(), [H, 2], tag="wen2")
            wst1 = ld(cp, wst1_d.ap(), [L + 6, H], tag="wst1")
            wst2 = ld(cp, wst2_d.ap(), [H, 9], tag="wst2")

            blat = ld(cp, blat_d.ap().rearrange("(h p) -> p h", p=128),
                      [128, 2], tag="blat")
            bnep = ld(cp, bnep_d.ap().rearrange("(h p) -> p h", p=128),
                      [128, 2], tag="bnep")
            bnd1 = ld(cp, bnd1_d.ap().rearrange("(h p) -> p h", p=H),
                      [H, 1], tag="bnd1")
            bnd2 = ld(cp, bnd2_d.ap().rearrange("(h p) -> p h", p=4),
                      [4, 1], tag="bnd2")
            bed1 = ld(cp, bed1_d.ap().rearrange("(h p) -> p h", p=H),
                      [H, 1], tag="bed1")
            bed2 = ld(cp, bed2_d.ap().rearrange("(h p) -> p h", p=3),
                      [3, 1], tag="bed2")
            ben1 = ld(cp, ben1_d.ap().rearrange("(h p) -> p h", p=H),
                      [H, 1], tag="ben1")
            ben2 = ld(cp, ben2_d.ap().rearrange("(h p) -> p h", p=2),
                      [2, 1], tag="ben2")
            bst1 = ld(cp, bst1_d.ap().rearrange("(h p) -> p h", p=H),
                      [H, 1], tag="bst1")
            bst2 = ld(cp, bst2_d.ap().rearrange("(h p) -> p h", p=9),
                      [9, 1], tag="bst2")

            t3_dram = dp.tile([NPAIR, TW], F32, tag="t3")

            # ---------- small precompute ----------
            # z_proj.T [2][128f, 256g] = relu(W_lat.T @ z.T + b_lat)
            zp = cp.tile([128, 2, B], F32, tag="zp")
            for fh in range(2):
                ps = pmisc.tile([128, B], F32, tag="mm")
                nc.tensor.matmul(ps[:], wlat[:, fh * 128:(fh + 1) * 128],
                                 zl[0:L, :], start=True, stop=True)
                nc.scalar.activation(zp[:, fh, :], ps[:], RELU,
                                     bias=blat[:, fh:fh + 1])
            # z_proj of this core's gs block [2][128f, 32]
            zpm = cp.tile([128, 2, GS_PER], F32, tag="zpm")
            for fh in range(2):
                ps = pmisc.tile([128, GS_PER], F32, tag="mm")
                nc.tensor.matmul(ps[:], wlat[:, fh * 128:(fh + 1) * 128],
                                 zmy[:], start=True, stop=True)
                nc.scalar.activation(zpm[:, fh, :], ps[:], RELU,
                                     bias=blat[:, fh:fh + 1])

            # P_bot.T [128h, 256g] (no bias)
            pbot = cp.tile([128, B], F32, tag="pbot")
            ps = pmisc.tile([128, B], F32, tag="mm")
            for fh in range(2):
                nc.tensor.matmul(ps[:], wbot[:, fh, :], zp[:, fh, :],
                                 start=(fh == 0), stop=(fh == 1))
            nc.scalar.copy(pbot[:], ps[:])

            # P_top(my gs)+b_ed1 [128h, 32]
            ptop = cp.tile([128, GS_PER], F32, tag="ptop")
            ps = pmisc.tile([128, GS_PER], F32, tag="mm")
            for fh in range(2):
                nc.tensor.matmul(ps[:], wtop[:, fh, :], zpm[:, fh, :],
                                 start=(fh == 0), stop=(fh == 1))
            nc.vector.tensor_scalar_add(ptop[:], ps[:], bed1[:, 0:1])

            # Q.T [128h, 256g] then dQ.T then dQ (g-major, 2 tiles)
            qt = cp.tile([128, B], F32, tag="qt")
            ps = pmisc.tile([128, B], F32, tag="mm")
            for fh in range(2):
                nc.tensor.matmul(ps[:], wnd1[:, fh, :], zp[:, fh, :],
                                 start=(fh == 0), stop=(fh == 1))
            nc.vector.tensor_copy(qt[:], ps[:])
            dqt = cp.tile([128, B], F32, tag="dqt")
            nc.vector.tensor_copy(dqt[:, 0:1], qt[:, 0:1])
            nc.vector.tensor_sub(dqt[:, 1:B], qt[:, 1:B], qt[:, 0:B - 1])
            dqg = cp.tile([128, 2, H], F32, tag="dqg")
            for gh in range(2):
                ps = pmisc.tile([128, H], F32, tag="mm")
                nc.tensor.transpose(ps[:], dqt[:, gh * 128:(gh + 1) * 128],
                                    ident[:])
                nc.vector.tensor_copy(dqg[:, gh, :], ps[:])

            # ---------- graph heads ----------
            for w1, b1, w2, b2, od, width in (
                (wen1, ben1, wen2, ben2, pe_d, 2),
                (wst1, bst1, wst2, bst2, ps_d, 9),
            ):
                psh = pmisc.tile([128, B], F32, tag="mm")
                nc.tensor.matmul(psh[:], w1[:], zl[:], start=True, stop=True)
                hh = wp.tile([128, B], F32, tag="hh")
                nc.scalar.activation(hh[:], psh[:], RELU, bias=b1[:, 0:1])
                pso = pmisc.tile([width, B], F32, tag="mm")
                nc.tensor.matmul(pso[:], w2[:, 0:width], hh[:],
                                 start=True, stop=True)
                ho = wp.tile([width, B], F32, tag="ho")
                nc.vector.tensor_scalar_add(ho[:], pso[:], b2[:, 0:1])
                nc.sync.dma_start(od.ap(), ho[:])

            # ---------- pair table build ----------
            t3pack = cp.tile([128, 2 * GS_PER, TW], F32, tag="t3pack")
            nc.vector.memset(t3pack[:], 0.0)
            for j in range(GS_PER):
                hp = ap.tile([128, B], F32, tag="hp")
                nc.scalar.activation(hp[:], pbot[:], RELU, bias=ptop[:, j:j + 1])
                ps3 = pmisc.tile([3, B], F32, tag="mm")
                nc.tensor.matmul(ps3[:], wed2[:, 0:3], hp[:],
                                 start=True, stop=True)
                t3c = ap.tile([3, B], F32, tag="t3c")
                nc.vector.tensor_scalar_add(t3c[:], ps3[:], bed2[:, 0:1])
                for gh in range(2):
                    pst = pmisc.tile([128, 3], F32, tag="mm")
                    nc.tensor.transpose(pst[:], t3c[:, gh * 128:(gh + 1) * 128],
                                        ident[0:3, 0:3])
                    nc.vector.tensor_copy(t3pack[:, 2 * j + gh, 0:3], pst[:])
            nc.sync.dma_start(
                t3_dram[:].rearrange("(s p) w -> p s w", p=128), t3pack[:])

            # ---------- edge gather ----------
            repack = cp.tile([128, NBLK, 3], F32, tag="repack")
            for t in range(NGATHER):
                gout = gp.tile([128, GCHB, TW], F32, tag="gout")
                nc.gpsimd.dma_gather(
                    gout[:], t3_dram[:],
                    eidx[:, t * (GCH // 16):(t + 1) * (GCH // 16)],
                    GCH, GCH, TW, single_packet=False)
                nc.vector.tensor_copy(
                    repack[:, t * GCHB:(t + 1) * GCHB, :], gout[:, :, 0:3])
            nc.sync.dma_start(re_d.ap(), repack[:].rearrange("p b t -> p (b t)"))

            # ---------- node path ----------
            for c in range(NCHUNK):
                net = nep_pool.tile([128, 8, CHUNK], F32, tag="net")
                nc.sync.dma_start(net[:], ne_d.ap()[c])
                nep = []
                for fh in range(2):
                    ps1 = p1p.tile([128, CHUNK], F32, tag="ps1")
                    for k in range(8):
                        nc.tensor.matmul(ps1[:], wnep[:, k, fh * 128:(fh + 1) * 128],
                                         net[:, k, :], start=(k == 0), stop=(k == 7))
                    nt = ap.tile([128, CHUNK], F32, tag=f"nep{fh}")
                    nc.scalar.activation(nt[:], ps1[:], RELU,
                                         bias=bnep[:, fh:fh + 1])
                    nep.append(nt)
                # staircase select matrices
                svs = []
                for gt in range(2):
                    sv = ap.tile([128, CHUNK], F32, tag=f"sv{gt}")
                    nc.vector.tensor_scalar(
                        sv[:], segb[:, c * CHUNK:(c + 1) * CHUNK],
                        iota[:, gt:gt + 1], None, mybir.AluOpType.is_ge)
                    svs.append(sv)
                ps2 = p2p.tile([128, CHUNK], F32, tag="ps2")
                for fh in range(2):
                    nc.tensor.matmul(ps2[:], wnd1[:, fh, :], nep[fh][:],
                                     start=(fh == 0), stop=False)
                for gt in range(2):
                    nc.tensor.matmul(ps2[:], dqg[:, gt, :], svs[gt][:],
                                     start=False, stop=(gt == 1))
                ht = ap.tile([128, CHUNK], F32, tag="ht")
                nc.scalar.activation(ht[:], ps2[:], RELU, bias=bnd1[:, 0:1])
                ps4 = p4p.tile([4, CHUNK], F32, tag="ps4")
                nc.tensor.matmul(ps4[:], wnd2[:, 0:4], ht[:],
                                 start=True, stop=True)
                rno = wp.tile([4, CHUNK], F32, tag="rno")
                nc.vector.tensor_scalar_add(rno[:], ps4[:], bnd2[:, 0:1])
                nc.sync.dma_start(rn_d.ap()[:, c * CHUNK:(c + 1) * CHUNK], rno[:])

    nc.compile()
    return nc


def _host_prep(inputs):
    f32 = np.float32
    z = np.ascontiguousarray(inputs["z"], f32)
    node_emb = np.ascontiguousarray(inputs["node_emb"], f32)
    lattice = np.ascontiguousarray(inputs["lattice"], f32)
    seg = np.ascontiguousarray(inputs["segment_ids"], np.int32)
    src = np.ascontiguousarray(inputs["src"], np.int32)
    dst = np.ascontiguousarray(inputs["dst"], np.int32)

    gs = seg[src]
    gd = seg[dst]
    core_of = gs // GS_PER
    pid = (gs % GS_PER).astype(np.int64) * B + gd
    order = np.lexsort((pid, core_of))
    counts = np.bincount(core_of, minlength=NCORES)
    starts = np.concatenate(([0], np.cumsum(counts)))

    zl = np.concatenate([z.T, np.tile(lattice[:, None], (1, B))], 0)
    zl = np.ascontiguousarray(zl, f32)
    iota = np.stack([np.arange(128), np.arange(128) + 128], 1)
    iota = np.ascontiguousarray(iota.astype(f32))
    ident = np.eye(128, dtype=f32)

    common = dict(
        zl=zl, iota=iota, ident=ident,
        wlat=inputs["W_lat"], blat=inputs["b_lat"],
        wnep=inputs["W_nep"], bnep=inputs["b_nep"],
        wnd1=inputs["W_nd1"], bnd1=inputs["b_nd1"],
        wnd2=inputs["W_nd2"], bnd2=inputs["b_nd2"],
        wtop=inputs["W_ed1"][:D2], wbot=inputs["W_ed1"][D2:],
        bed1=inputs["b_ed1"], wed2=inputs["W_ed2"], bed2=inputs["b_ed2"],
        wen1=inputs["W_en1"], ben1=inputs["b_en1"],
        wen2=inputs["W_en2"], ben2=inputs["b_en2"],
        wst1=inputs["W_st1"], bst1=inputs["b_st1"],
        wst2=inputs["W_st2"], bst2=inputs["b_st2"],
    )
    common = {k: np.ascontiguousarray(v, f32) for k, v in common.items()}
    common["iota"] = iota

    in_maps = []
    for c in range(NCORES):
        slab = node_emb[c * NODES_PER:(c + 1) * NODES_PER].T  # [1024, 8192]
        ne = np.ascontiguousarray(
            slab.reshape(8, 128, NCHUNK, CHUNK).transpose(2, 1, 0, 3))
        seg_c = seg[c * NODES_PER:(c + 1) * NODES_PER].astype(ml_dtypes.bfloat16)
        segb = np.ascontiguousarray(np.tile(seg_c[None, :], (128, 1)))
        sel = order[starts[c]:starts[c + 1]]
        assert len(sel) <= EDGE_CAP, f"edge bucket overflow: {len(sel)}"
        pid_pad = np.zeros(EDGE_CAP, np.int16)
        pid_pad[:len(sel)] = pid[sel].astype(np.int16)
        eidx = np.ascontiguousarray(
            np.tile(pid_pad.reshape(EDGE_CAP // 16, 16).T, (8, 1)))
        zmy = np.ascontiguousarray(z.T[:, c * GS_PER:(c + 1) * GS_PER], f32)
        in_maps.append(dict(common, ne=ne, segb=segb, eidx=eidx, zmy=zmy))
    return in_maps, order, starts


def kernel(**inputs):
    if "nc" not in _cached:
        _cached["nc"] = _build()
    nc = _cached["nc"]
    in_maps, order, starts = _host_prep(inputs)
    res = run_bass_kernel_spmd(nc, in_maps, core_ids=list(range(NCORES)))
    rs = res.results

    rn = np.concatenate([rs[c]["rn"].T for c in range(NCORES)], 0)
    re = np.empty((E, 3), np.float32)
    for c in range(NCORES):
        sel = order[starts[c]:starts[c + 1]]
        rp = rs[c]["re"].reshape(128, NBLK, 3).transpose(1, 0, 2).reshape(-1, 3)
        re[sel] = rp[:len(sel)]
    pe = rs[0]["pe"].T.copy()
    ps = rs[0]["ps"].T.copy()
    return rn, re, pe, ps
